# revision 1
# baseline (speedup 1.0000x reference)
"""Trainium2 Bass kernel for GQA attention (RoPE + QK-L2norm + causal + wo). v2.

Full-problem shapes: B=2, T=2048, DIM=2048, H=32 q-heads, G=8 kv-heads, D=64.
Sharding over 8 cores: core = (batch b = c//4, kv-pair p = c%4) ->
8 q heads / 2 kv heads per core. Each core computes a partial (feature-major)
output out_T = wo_slice @ y_slice of shape [DIM, T]; host sums 4 partials per
batch and transposes.

v2 redesign vs baseline:
- attention y = p@v flipped: out [128 tokens x 65] so each matmul pays 65
  cycles instead of 512 for a 65-partition output; fully-masked tiles skipped.
- y relayout to feature-major via DMA xbar transpose (off the PE).
- softmax denominator reciprocal applied via per-partition tensor_scalar
  (tokens now on partitions) - kills the DRAM broadcast round-trip.
- rope chain in bf16 SBUF (DVE 2x modes), adds on gpsimd.
- wo matmuls software-pipelined one chunk behind attention as PE filler
  during exp waits; output DMA'd f32 directly from PSUM.
"""
import math
from contextlib import ExitStack

import numpy as np
import ml_dtypes

import concourse.bass as bass
import concourse.tile as tile
from concourse import mybir

F32 = mybir.dt.float32
BF16 = mybir.dt.bfloat16


# ---------------------------------------------------------------- host prep
def _head_perm(D=64):
    """Permutation: new position j (0..63) -> original dim index."""
    perm = np.empty(D, dtype=np.int64)
    for j in range(D):
        block, q = j // 32, j % 32
        freq = (q % 16) + 16 * block
        perm[j] = 2 * freq + (1 if q >= 16 else 0)
    return perm


def _freq_of_partition(P=128):
    fr = np.empty(P, dtype=np.int64)
    sign = np.empty(P, dtype=np.float32)
    for p in range(P):
        fr[p] = (p % 16) + 16 * ((p % 64) // 32)
        sign[p] = -1.0 if (p % 32) < 16 else 1.0
    return fr, sign


def prep_core_inputs(x, freqs_cos, freqs_sin, wq, wk, wv, wo, q_scale, k_scale,
                     T=2048, DIM=2048, D=64):
    """Returns list of 8 in_maps (numpy arrays, keyed by dram tensor names)."""
    bf = ml_dtypes.bfloat16
    perm = _head_perm(D)
    fr, sign = _freq_of_partition(128)

    Cb = freqs_cos[:, fr].T.astype(bf).copy()                    # [128, T]
    Sb = (sign[:, None] * freqs_sin[:, fr].T).astype(bf).copy()  # [128, T]

    r, c = np.mgrid[0:128, 0:128]
    mask01 = np.where(c >= r, 1.0, 0.0).astype(bf)

    selq = np.zeros((128, 4, 8), np.float32)
    for m in range(4):
        selq[0:64, m, 2 * m] = 1.0
        selq[64:128, m, 2 * m + 1] = 1.0
    selq = selq.reshape(128, 32).astype(bf).copy()
    selk = np.zeros((128, 2), bf)
    selk[0:64, 0] = 1.0
    selk[64:128, 1] = 1.0
    sel2q = np.zeros((8, 4, 128), np.float32)
    for m in range(4):
        sel2q[2 * m, m, 0:64] = 1.0
        sel2q[2 * m + 1, m, 64:128] = 1.0
    sel2q = sel2q.reshape(8, 512).astype(bf).copy()
    sel2k = np.zeros((2, 128), bf)
    sel2k[0, 0:64] = 1.0
    sel2k[1, 64:128] = 1.0
    ident = np.eye(128, dtype=np.float32).astype(bf)

    qs = np.asarray(q_scale).reshape(-1)   # [32]
    ks = np.asarray(k_scale).reshape(-1)   # [8]

    xT = {b: np.ascontiguousarray(x[b].T).astype(bf) for b in range(x.shape[0])}

    in_maps = []
    for core in range(8):
        b, p = core // 4, core % 4
        # wq rows for this core, permuted per head, transposed
        wq_rows = wq[512 * p: 512 * (p + 1), :].reshape(8, D, DIM)[:, perm, :]
        wqT = np.ascontiguousarray(wq_rows.reshape(512, DIM).T).astype(bf)
        wk_rows = wk[128 * p: 128 * (p + 1), :].reshape(2, D, DIM)[:, perm, :]
        wkT = np.ascontiguousarray(wk_rows.reshape(128, DIM).T).astype(bf)
        wvT = np.ascontiguousarray(wv[128 * p: 128 * (p + 1), :].T).astype(bf)
        woT = np.ascontiguousarray(wo[:, 512 * p: 512 * (p + 1)].T).astype(bf)

        fh = np.empty((8, 1), np.float32)
        for h in range(8):
            fh[h, 0] = (qs[8 * p + h] * ks[2 * p + h // 4] / math.sqrt(D)) ** 2

        in_maps.append({
            "xT": xT[b], "wqT": wqT, "wkT": wkT, "wvT": wvT, "woT": woT,
            "Cb": Cb, "Sb": Sb, "mask01": mask01, "fsq": fh, "ident": ident,
            "selq": selq, "selk": selk, "sel2q": sel2q, "sel2k": sel2k,
        })
    return in_maps


# ---------------------------------------------------------------- device code
def build_nc(T=2048, DIM=2048):
    """Build the per-core Bass program (identical for all 8 cores)."""
    NT = T // 512          # number of 512-wide token chunks
    ND = DIM // 128        # contraction tiles
    NTT = T // 128         # token tiles
    nc = bass.Bass("TRN2", target_bir_lowering=False, debug=False)

    xT = nc.dram_tensor("xT", [DIM, T], BF16, kind="ExternalInput").ap()
    wqT = nc.dram_tensor("wqT", [DIM, 512], BF16, kind="ExternalInput").ap()
    wkT = nc.dram_tensor("wkT", [DIM, 128], BF16, kind="ExternalInput").ap()
    wvT = nc.dram_tensor("wvT", [DIM, 128], BF16, kind="ExternalInput").ap()
    woT = nc.dram_tensor("woT", [512, DIM], BF16, kind="ExternalInput").ap()
    Cb = nc.dram_tensor("Cb", [128, T], BF16, kind="ExternalInput").ap()
    Sb = nc.dram_tensor("Sb", [128, T], BF16, kind="ExternalInput").ap()
    mask01 = nc.dram_tensor("mask01", [128, 128], BF16, kind="ExternalInput").ap()
    fsq = nc.dram_tensor("fsq", [8, 1], F32, kind="ExternalInput").ap()
    selq = nc.dram_tensor("selq", [128, 32], BF16, kind="ExternalInput").ap()
    selk = nc.dram_tensor("selk", [128, 2], BF16, kind="ExternalInput").ap()
    sel2q = nc.dram_tensor("sel2q", [8, 512], BF16, kind="ExternalInput").ap()
    sel2k = nc.dram_tensor("sel2k", [2, 128], BF16, kind="ExternalInput").ap()
    ident = nc.dram_tensor("ident", [128, 128], BF16, kind="ExternalInput").ap()
    outT = nc.dram_tensor("outT", [DIM, T], BF16, kind="ExternalOutput").ap()

    SHUF = [(i + 16) % 32 for i in range(32)]
    EXP = mybir.ActivationFunctionType.Exp
    SQRT = mybir.ActivationFunctionType.Sqrt
    COPY = mybir.ActivationFunctionType.Copy
    MUL = mybir.AluOpType.mult
    ADD = mybir.AluOpType.add

    with tile.TileContext(nc) as tc, ExitStack() as ctx:
        const = ctx.enter_context(tc.tile_pool(name="const", bufs=1))
        persist = ctx.enter_context(tc.tile_pool(name="persist", bufs=1))

        c_m01 = const.tile([128, 128], BF16)
        c_fsq = const.tile([8, 1], F32)
        c_selq = const.tile([128, 4, 8], BF16)
        c_selk = const.tile([128, 2], BF16)
        c_sel2q = const.tile([8, 4, 128], BF16)
        c_sel2k = const.tile([2, 128], BF16)
        c_id = const.tile([128, 128], BF16)
        c_C = const.tile([128, T], BF16)
        c_S = const.tile([128, T], BF16)

        # persistent tensors
        xqp = ctx.enter_context(tc.tile_pool(name="xq", bufs=2))
        wq_sb = persist.tile([128, ND, 512], BF16)
        wk_sb = persist.tile([128, ND, 128], BF16)
        wv_sb = persist.tile([128, ND, 128], BF16)
        wo_sb = persist.tile([128, 4, DIM], BF16)
        qhat = persist.tile([128, 4, T], BF16)
        khatA = persist.tile([128, T], BF16)
        khatB = persist.tile([128, T], BF16)
        vslab = persist.tile([128, NTT, 130], BF16)

        # input DMAs, ordered for earliest first-matmul: x tiles stream on SP,
        # weights/consts stream on Act. Q(c0) runs d-outer paced by the x DMAs.
        xq = {}

        def load_xq(c):
            xq[c] = xqp.tile([128, ND, 512], BF16, tag="xq", name=f"xq{c}")
            for d in range(ND):
                nc.sync.dma_start(out=xq[c][:, d, :],
                                  in_=xT[128 * d:128 * (d + 1),
                                         512 * c:512 * (c + 1)])

        xq[0] = xqp.tile([128, ND, 512], BF16, tag="xq", name="xq0")
        xq[1] = xqp.tile([128, ND, 512], BF16, tag="xq", name="xq1")
        for d in range(ND):
            for c in (0, 1):
                nc.sync.dma_start(out=xq[c][:, d, :],
                                  in_=xT[128 * d:128 * (d + 1),
                                         512 * c:512 * (c + 1)])
        for m in range(4):
            nc.scalar.dma_start(out=wq_sb[:, :, 128 * m:128 * (m + 1)],
                                in_=wqT.rearrange("(d p) f -> p d f", p=128)
                                [:, :, 128 * m:128 * (m + 1)])
        for h in range(2):
            nc.scalar.dma_start(out=c_C[:, 1024 * h:1024 * (h + 1)],
                                in_=Cb[:, 1024 * h:1024 * (h + 1)])
            nc.scalar.dma_start(out=c_S[:, 1024 * h:1024 * (h + 1)],
                                in_=Sb[:, 1024 * h:1024 * (h + 1)])
        nc.scalar.dma_start(out=c_fsq[:], in_=fsq)
        nc.scalar.dma_start(out=c_selq[:], in_=selq.rearrange("p (m h) -> p m h", m=4))
        nc.scalar.dma_start(out=c_sel2q[:], in_=sel2q.rearrange("h (m p) -> h m p", m=4))
        wkr = wkT.rearrange("(d p) f -> p d f", p=128)
        wvr = wvT.rearrange("(d p) f -> p d f", p=128)
        for h in range(2):
            nc.scalar.dma_start(out=wk_sb[:, 8 * h:8 * (h + 1), :],
                                in_=wkr[:, 8 * h:8 * (h + 1), :])
        for h in range(2):
            nc.scalar.dma_start(out=wv_sb[:, 8 * h:8 * (h + 1), :],
                                in_=wvr[:, 8 * h:8 * (h + 1), :])
        nc.scalar.dma_start(out=c_selk[:], in_=selk)
        nc.scalar.dma_start(out=c_sel2k[:], in_=sel2k)
        nc.scalar.dma_start(out=c_id[:], in_=ident)
        nc.scalar.dma_start(out=c_m01[:], in_=mask01)
        wor = woT.rearrange("(t p) f -> p t f", p=128)
        for yt in range(4):
            nc.scalar.dma_start(out=wo_sb[:, yt, :], in_=wor[:, yt, :])

        nc.vector.memset(vslab[:, :, 64:65], 1.0)
        nc.vector.memset(vslab[:, :, 129:130], 1.0)
        mb2 = bass.AP(tensor=c_m01.tensor, offset=c_m01[:].offset,
                      ap=[list(c_m01[:].ap[0]), [0, 2], list(c_m01[:].ap[1])])

        # ---------------- Phase A: projections + rope + norm ----------------
        with tc.tile_pool(name="nm_ps", bufs=1, space="PSUM") as nm_ps, \
             tc.tile_pool(name="bc_ps", bufs=1, space="PSUM") as bc_ps, \
             tc.tile_pool(name="ropesb", bufs=4) as rsb, \
             tc.tile_pool(name="rq", bufs=4) as rqp, \
             tc.tile_pool(name="sq", bufs=5) as sqp, \
             tc.tile_pool(name="small", bufs=2) as small:

            def rope_chain(pp, cs):
                """Common rope on a projected psum tile; returns (rot, sqt)."""
                sb = rsb.tile([128, 512], BF16, tag="sb")
                nc.scalar.activation(out=sb[:], in_=pp[:], func=COPY)
                sw = rsb.tile([128, 512], BF16, tag="sw")
                nc.vector.stream_shuffle(out=sw[:], in_=sb[:], mask=SHUF)
                u = rsb.tile([128, 512], BF16, tag="u")
                nc.vector.tensor_tensor(out=u[:], in0=sb[:], in1=c_C[:, cs], op=MUL)
                t2 = rsb.tile([128, 512], BF16, tag="t2")
                nc.vector.tensor_tensor(out=t2[:], in0=sw[:], in1=c_S[:, cs], op=MUL)
                rot = rqp.tile([128, 512], BF16, tag="rot")
                nc.gpsimd.tensor_tensor(out=rot[:], in0=u[:], in1=t2[:], op=ADD)
                sqt = sqp.tile([128, 512], BF16, tag="sq")
                nc.vector.tensor_tensor(out=sqt[:], in0=sb[:], in1=sb[:], op=MUL)
                return rot, sqt

            def q_norm_tail(c, cs, sq_tiles, rq_tiles):
                nm = nm_ps.tile([8, 512], F32, tag="nm", name=f"nm{c}")
                for m in range(4):
                    nc.tensor.matmul(nm[:], c_selq[:, m, :], sq_tiles[m][:],
                                     start=(m == 0), stop=(m == 3))
                rn = small.tile([8, 512], F32, tag="rn", name=f"rn{c}")
                nc.vector.reciprocal(out=rn[:], in_=nm[:])
                fac = small.tile([8, 512], BF16, tag="fac", name=f"fac{c}")
                nc.scalar.activation(out=fac[:], in_=rn[:], func=SQRT, scale=c_fsq[:])
                for m in range(4):
                    bc = bc_ps.tile([128, 512], F32, tag="bc", name=f"bc{c}_{m}")
                    nc.tensor.matmul(bc[:], c_sel2q[:, m, :], fac[:], start=True, stop=True)
                    nc.vector.tensor_tensor(out=qhat[:, m, cs], in0=rq_tiles[m][:],
                                            in1=bc[:], op=MUL)

            # S0: Q(c0) m0-3 + K0 + V0, six accumulators d-outer, x-DMA paced
            sb_k0 = sb_v0 = None
            with tc.tile_pool(name="q0_ps", bufs=6, space="PSUM") as q0_ps:
                cs0 = slice(0, 512)
                qps0 = [q0_ps.tile([128, 512], F32, tag="pp", name=f"qp0_{m}")
                        for m in range(4)]
                kp0 = q0_ps.tile([128, 512], F32, tag="pp", name="kp0")
                vf0 = q0_ps.tile([128, 512], F32, tag="pp", name="vf0")
                for d in range(ND):
                    for m in range(4):
                        nc.tensor.matmul(qps0[m][:], wq_sb[:, d, 128 * m:128 * (m + 1)],
                                         xq[0][:, d, :], start=(d == 0), stop=(d == ND - 1))
                    nc.tensor.matmul(kp0[:], wk_sb[:, d, :], xq[0][:, d, :],
                                     start=(d == 0), stop=(d == ND - 1))
                    nc.tensor.matmul(vf0[:], wv_sb[:, d, :], xq[0][:, d, :],
                                     start=(d == 0), stop=(d == ND - 1))
                sq0, rq0 = [], []
                for m in range(4):
                    rq, sqt = rope_chain(qps0[m], cs0)
                    sq0.append(sqt)
                    rq0.append(rq)
                q_norm_tail(0, cs0, sq0, rq0)
                sb_k0 = rope_chain(kp0, cs0)     # (rk, sqk)
                vfs0 = rsb.tile([128, 512], BF16, tag="vfs", name="vfs0")
                nc.scalar.activation(out=vfs0[:], in_=vf0[:], func=COPY)

            # chunk-major: K(c), V(c), then Q(c+1)
            with tc.tile_pool(name="proj_ps", bufs=2, space="PSUM") as proj_ps, \
                 tc.tile_pool(name="vt_ps", bufs=2, space="PSUM") as vt_ps:
                for c in range(NT):
                    cs = slice(512 * c, 512 * (c + 1))
                    if c == 0:
                        rk, sqk = sb_k0
                        vfs = vfs0
                    else:
                        kp = proj_ps.tile([128, 512], F32, tag="pp", name=f"kp{c}")
                        for d in range(ND):
                            nc.tensor.matmul(kp[:], wk_sb[:, d, :], xq[c][:, d, :],
                                             start=(d == 0), stop=(d == ND - 1))
                        vf = proj_ps.tile([128, 512], F32, tag="pp", name=f"vf{c}")
                        for d in range(ND):
                            nc.tensor.matmul(vf[:], wv_sb[:, d, :], xq[c][:, d, :],
                                             start=(d == 0), stop=(d == ND - 1))
                        rk, sqk = rope_chain(kp, cs)
                        vfs = rsb.tile([128, 512], BF16, tag="vfs", name=f"vfs{c}")
                        nc.scalar.activation(out=vfs[:], in_=vf[:], func=COPY)
                    nmk = nm_ps.tile([2, 512], F32, tag="nm", name=f"nmk{c}")
                    nc.tensor.matmul(nmk[:], c_selk[:], sqk[:], start=True, stop=True)
                    rnk = small.tile([2, 512], F32, tag="rnk", name=f"rnk{c}")
                    nc.vector.reciprocal(out=rnk[:], in_=nmk[:])
                    fack = small.tile([2, 512], BF16, tag="fack", name=f"fack{c}")
                    nc.scalar.activation(out=fack[:], in_=rnk[:], func=SQRT)
                    bck = bc_ps.tile([128, 512], F32, tag="bc", name=f"bck{c}")
                    nc.tensor.matmul(bck[:], c_sel2k[:], fack[:], start=True, stop=True)
                    nc.vector.tensor_tensor(out=khatA[0:64, cs], in0=rk[0:64, :],
                                            in1=bck[0:64, :], op=MUL)
                    nc.vector.tensor_tensor(out=khatB[64:128, cs], in0=rk[64:128, :],
                                            in1=bck[64:128, :], op=MUL)
                    nc.sync.dma_start(out=khatA[64:128, cs], in_=khatA[0:64, cs])
                    nc.sync.dma_start(out=khatB[0:64, cs], in_=khatB[64:128, cs])

                    # V: PE-transpose to token-major
                    for q4 in range(4):
                        tt = 4 * c + q4
                        vt = vt_ps.tile([128, 128], BF16, tag="vt", name=f"vt{c}_{q4}")
                        nc.tensor.transpose(vt[:], vfs[:, 128 * q4:128 * (q4 + 1)], c_id[:])
                        nc.vector.tensor_copy(out=vslab[:, tt, 0:64], in_=vt[:, 0:64])
                        nc.vector.tensor_copy(out=vslab[:, tt, 65:129], in_=vt[:, 64:128])

                    # Q(c+1)
                    if c + 1 < NT:
                        if c + 2 < NT:
                            load_xq(c + 2)
                        cq = c + 1
                        csq = slice(512 * cq, 512 * (cq + 1))
                        sq_tiles, rq_tiles = [], []
                        for m in range(4):
                            qp = proj_ps.tile([128, 512], F32, tag="pp",
                                              name=f"qp{cq}_{m}")
                            for d in range(ND):
                                nc.tensor.matmul(qp[:], wq_sb[:, d, 128 * m:128 * (m + 1)],
                                                 xq[cq][:, d, :],
                                                 start=(d == 0), stop=(d == ND - 1))
                            rq, sqt = rope_chain(qp, csq)
                            sq_tiles.append(sqt)
                            rq_tiles.append(rq)
                        q_norm_tail(cq, csq, sq_tiles, rq_tiles)

        # ---------------- Phase B: attention + output projection ----------------
        with tc.tile_pool(name="pT", bufs=9) as pTp, \
             tc.tile_pool(name="yn", bufs=8) as ynp, \
             tc.tile_pool(name="yT2", bufs=2) as yT2p, \
             tc.tile_pool(name="rden", bufs=4) as rdnp, \
             tc.tile_pool(name="ostage", bufs=4) as ostg, \
             tc.tile_pool(name="s_ps", bufs=2, space="PSUM") as s_ps, \
             tc.tile_pool(name="yA_ps", bufs=1, space="PSUM") as yA_ps, \
             tc.tile_pool(name="yB_ps", bufs=1, space="PSUM") as yB_ps, \
             tc.tile_pool(name="o_ps", bufs=2, space="PSUM") as o_ps:

            def wo_tile(yT2c, csp, mo):
                op = o_ps.tile([128, 512], F32, tag="op", name=f"op_{mo}")
                for yt in range(4):
                    nc.tensor.matmul(op[:], wo_sb[:, yt, 128 * mo:128 * (mo + 1)],
                                     yT2c[:, :, yt, :], start=(yt == 0), stop=(yt == 3))
                ost = ostg.tile([128, 512], BF16, tag="ost", name=f"ost_{mo}")
                nc.vector.tensor_copy(out=ost[:], in_=op[:])
                nc.sync.dma_start(out=outT[128 * mo:128 * (mo + 1), csp], in_=ost[:])

            prev = None      # (yT2 tile, cs slice) of previous chunk
            for c in range(NT):
                jmax = 4 * c + 3
                cs = slice(512 * c, 512 * (c + 1))
                yn_tiles = [ynp.tile([128, 512], BF16, tag="yn", name=f"yn{c}_{i}")
                            for i in range(4)]
                for hp in range(4):
                    kd = khatA if hp < 2 else khatB
                    m = hp
                    ypA = yA_ps.tile([128, 4, 65], F32, tag="ypA", name=f"ypA{c}_{hp}")
                    ypB = yB_ps.tile([128, 4, 65], F32, tag="ypB", name=f"ypB{c}_{hp}")
                    nc.vector.memset(ypA[:], 0.0)
                    nc.vector.memset(ypB[:], 0.0)
                    def y_mms(j, pT):
                        vck = slice(0, 65) if hp < 2 else slice(65, 130)
                        for hi, yp, vc in ((0, ypA, vck), (1, ypB, vck)):
                            for q4 in range(4):
                                tt = 4 * c + q4
                                if j > tt:
                                    continue  # fully masked tile
                                nc.tensor.matmul(
                                    yp[:, q4, :],
                                    pT[:, hi, 128 * q4:128 * (q4 + 1)],
                                    vslab[:, j, vc],
                                    start=False, stop=(j == tt),
                                    skip_group_check=True)

                    # interleaved: scores(j)/exp(j) ... y(j-1), paced wo tiles
                    pT_prev = None
                    wo_done = 0
                    for j in range(jmax + 1):
                        o = max(0, 128 * j - 512 * c)
                        sps = s_ps.tile([128, 2, 512], F32, tag="sps",
                                        name=f"sps{c}_{hp}_{j}")
                        for hi in (0, 1):
                            b = 64 * hi
                            nc.tensor.matmul(
                                sps[:, hi, o:512],
                                kd[b:b + 64, 128 * j:128 * (j + 1)],
                                qhat[b:b + 64, m, 512 * c + o: 512 * (c + 1)],
                                start=True, stop=True)
                        pT = pTp.tile([128, 2, 512], BF16, tag="pT",
                                      name=f"pT{c}_{hp}_{j}")
                        nc.scalar.activation(out=pT[:, :, o:512],
                                             in_=sps[:, :, o:512], func=EXP)
                        if 128 * j >= 512 * c:  # diagonal block: causal 0/1 mask
                            nc.vector.tensor_tensor(out=pT[:, :, o:o + 128],
                                                    in0=pT[:, :, o:o + 128],
                                                    in1=mb2, op=MUL)
                        # paced PE filler: wo of previous chunk (4 tiles per hp)
                        if prev is not None and (j + 1) % (c + 1) == 0 and wo_done < 4:
                            wo_tile(prev[0], prev[1], 4 * hp + wo_done)
                            wo_done += 1
                        if pT_prev is not None:
                            y_mms(j - 1, pT_prev)
                        pT_prev = pT
                    y_mms(jmax, pT_prev)
                    while prev is not None and wo_done < 4:
                        wo_tile(prev[0], prev[1], 4 * hp + wo_done)
                        wo_done += 1

                    # normalize: rden per token, then scale into yn staging
                    for hi, yp in ((0, ypA), (1, ypB)):
                        rdn = rdnp.tile([128, 4, 1], F32, tag="rdn",
                                        name=f"rdn{c}_{hp}_{hi}")
                        nc.vector.reciprocal(out=rdn[:], in_=yp[:, :, 64:65])
                        for q4 in range(4):
                            nc.vector.tensor_scalar(
                                out=yn_tiles[q4][:, 128 * hp + 64 * hi:
                                                 128 * hp + 64 * hi + 64],
                                in0=yp[:, q4, 0:64],
                                scalar1=rdn[:, q4, :], scalar2=None, op0=MUL)

                # relayout y to feature-major via DMA xbar transpose
                yT2 = yT2p.tile([128, 4, 4, 128], BF16, tag="yT2", name=f"yT2_{c}")
                for q4 in range(4):
                    nc.sync.dma_start_transpose(out=yT2[:, q4, :, :], in_=yn_tiles[q4][:])
                prev = (yT2, cs)

            if True:
                for mo in range(16):
                    op = o_ps.tile([128, 512], F32, tag="op", name=f"op2_{mo}")
                    for yt in range(4):
                        nc.tensor.matmul(op[:], wo_sb[:, yt, 128 * mo:128 * (mo + 1)],
                                         prev[0][:, :, yt, :],
                                         start=(yt == 0), stop=(yt == 3))
                    ost = ostg.tile([128, 512], BF16, tag="ost", name=f"ost3_{mo}")
                    if mo % 2 == 0:
                        nc.vector.tensor_copy(out=ost[:], in_=op[:])
                    else:
                        nc.scalar.activation(out=ost[:], in_=op[:], func=COPY)
                    nc.sync.dma_start(out=outT[128 * mo:128 * (mo + 1), prev[1]],
                                      in_=ost[:])
    return nc


def postprocess(results, B=2, T=2048, DIM=2048):
    out = np.empty((B, T, DIM), np.float32)
    for b in range(B):
        acc = results[4 * b]["outT"].astype(np.float32)
        for i in range(1, 4):
            acc = acc + results[4 * b + i]["outT"]
        out[b] = acc.T
    return out


# ------------- multi-wait splitting (neuronxcc single-wait limit) -------------
def split_multi_waits(nc):
    for f in nc.m.functions:
        for blk in f.blocks:
            insts = list(blk.instructions)
            changed = False
            out = []
            for inst in insts:
                si = getattr(inst, "sync_info", None)
                if si is not None and len(si.on_wait) > 1:
                    waits = list(si.on_wait)
                    for j, w in enumerate(waits[:-1]):
                        d = mybir.InstDrain(name=f"{inst.name}-sw{j}", ins=[], outs=[])
                        d.engine = inst.engine
                        d.sync_info = mybir.SyncInfo(on_wait=[w], on_update=[])
                        out.append(d)
                    inst.sync_info = mybir.SyncInfo(
                        on_wait=[waits[-1]], on_update=list(si.on_update)
                    )
                    changed = True
                out.append(inst)
            if changed:
                blk.instructions = out


# ---------------------------------------------------------------- entry point
_CACHE = {}


def kernel(x, freqs_cos, freqs_sin, wq, wk, wv, wo, q_scale, k_scale):
    """Full-input GQA attention on 8 NeuronCores; returns [2, 2048, 2048] f32."""
    from concourse.bass_utils import run_bass_kernel_spmd

    x = np.asarray(x, dtype=np.float32)
    freqs_cos = np.asarray(freqs_cos, dtype=np.float32)
    freqs_sin = np.asarray(freqs_sin, dtype=np.float32)
    wq = np.asarray(wq, dtype=np.float32)
    wk = np.asarray(wk, dtype=np.float32)
    wv = np.asarray(wv, dtype=np.float32)
    wo = np.asarray(wo, dtype=np.float32)

    if "nc" not in _CACHE:
        nc = build_nc(T=2048, DIM=2048)
        split_multi_waits(nc)
        _CACHE["nc"] = nc
    nc = _CACHE["nc"]

    in_maps = prep_core_inputs(x, freqs_cos, freqs_sin, wq, wk, wv, wo,
                               q_scale, k_scale, T=2048, DIM=2048)
    res = run_bass_kernel_spmd(nc, in_maps, core_ids=list(range(8)))
    return postprocess(res.results)

